# revision 6
# baseline (speedup 1.0000x reference)
"""ClusterAttention Trainium2 kernel.

Sharding: 48 (b*h) rows -> 6 rows per core (8 cores). Host gathers feat into
cluster order per row (transposed, c-major) as shard prep; device does all
matmul/softmax compute; host scatters head outputs back to token order between
the two device phases and sums nothing (phase B consumes all 12 heads per
token on one core).

Phase A (per core, per row r with head h):
  - o-major qk projection: psum[52,512] = wqk[cc].T @ featT tile, accumulated
    over 3 c-chunks. M-layout: rows 0:16 q*scale, 16 qA, 32:48 k, 51 kB.
  - q_sb[20,8192] rows: [q(16), qA, 1, -s, 1];  k_sb rows: [k(16), 1, s+b+c, 1, kB]
    so that sum_p q_aug[p]*k_aug[p] = scale*(q+bq).(k+bk) + s_j - s_i + b_pos.
  - t-major v projection: psum[128,256] = featT_chunk.T @ wv[cc], -> v_sb with a
    ones column per 64-col chunk (for the softmax denominator).
  - per cluster: S'[j,i] = k_aug.T@q_aug (K=20), E = exp(S'), AV: out[i,0:65] =
    sum_j E[j,i-chunk].T @ v_aug[j,0:65]; col 64 = denominator. Normalize by
    DVE reciprocal + per-partition scalar multiply. DMA out rows.

Phase B (per core): o-major projection outT[o,t] = w_proj chunks.T @ feat2T,
bias added via DVE tensor_scalar.
"""
import os
import numpy as np
import ml_dtypes

import concourse.bacc as bacc
import concourse.tile as tile
from concourse import mybir
from concourse.bass_utils import run_bass_kernel_spmd

B, N, C, H, D, K, M = 4, 8192, 384, 12, 2, 32, 256
CH = C // H // 2            # 16
BH = B * H                  # 48
R = BH // 8                 # 6 rows per core
SCALE = float((C // H) ** -0.5)
NT = N // 512               # 16 token tiles per row
TPB = N * B // 8            # 4096 tokens per core in phase B

_DT = {"f32": mybir.dt.float32, "bf16": mybir.dt.bfloat16}
_NP = {"f32": np.float32, "bf16": ml_dtypes.bfloat16}


def _parse_cfg():
    s = os.environ.get("KCFG", "feat=f32,qk=f32,e=f32,b=f32,r=0")
    cfg = {}
    for part in s.split(","):
        k, v = part.split("=")
        cfg[k] = v
    cfg.setdefault("feat", "f32"); cfg.setdefault("qk", "f32")
    cfg.setdefault("e", "f32"); cfg.setdefault("b", "f32")
    cfg.setdefault("r", "0")  # r=1: bitcast f32 matmul operands to float32r
    return cfg


CFG = _parse_cfg()


def _mm(ap, kind):
    # bitcast fp32 matmul inputs to float32r when enabled
    if CFG["r"] == "1" and CFG[kind] == "f32":
        return ap.bitcast(mybir.dt.float32r)
    return ap


def build_phase_a():
    dt_feat, dt_qk, dt_e = _DT[CFG["feat"]], _DT[CFG["qk"]], _DT[CFG["e"]]
    nc = bacc.Bacc(None, target_bir_lowering=False)
    featT = nc.dram_tensor("featT", [R * 3 * 128, N], dt_feat, kind="ExternalInput")
    wqk = nc.dram_tensor("wqk", [R * 3 * 128, 52], dt_feat, kind="ExternalInput")
    wv = nc.dram_tensor("wv", [R * 3 * 128, 64], dt_feat, kind="ExternalInput")
    aux = nc.dram_tensor("aux", [R * 8, N], dt_qk, kind="ExternalInput")
    out_g = nc.dram_tensor("out_g", [R * N, 64], mybir.dt.float32, kind="ExternalOutput")

    with tile.TileContext(nc) as tc:
        with (
            tc.tile_pool(name="sb_feat", bufs=1) as p_feat,
            tc.tile_pool(name="sb_row", bufs=1) as p_row,
            tc.tile_pool(name="sb_w", bufs=2) as p_w,
            tc.tile_pool(name="sb_e", bufs=4) as p_e,
            tc.tile_pool(name="sb_out", bufs=4) as p_out,
            tc.tile_pool(name="ps_qkp", bufs=2, space="PSUM") as ps_qkp,
            tc.tile_pool(name="ps_sp", bufs=2, space="PSUM") as ps_sp,
            tc.tile_pool(name="ps_vp", bufs=2, space="PSUM") as ps_vp,
            tc.tile_pool(name="ps_op", bufs=2, space="PSUM") as ps_op,
        ):
            for r in range(R):
                fb = r * 3 * 128
                # per-row persistent tiles
                ft = p_feat.tile([128, 3 * N], dt_feat, tag="ft")
                q_sb = p_row.tile([20, N], dt_qk, tag="q_sb")
                k_sb = p_row.tile([20, N], dt_qk, tag="k_sb")
                v_sb = p_row.tile([128, 64 * 65], dt_e, tag="v_sb")
                wqk_sb = p_w.tile([128, 3 * 52], dt_feat, tag="wqk_sb")
                wv_sb = p_w.tile([128, 3 * 64], dt_feat, tag="wv_sb")
                for cc in range(3):
                    nc.sync.dma_start(ft[:, cc * N:(cc + 1) * N],
                                      featT[fb + cc * 128: fb + (cc + 1) * 128, :])
                    nc.sync.dma_start(wqk_sb[:, cc * 52:(cc + 1) * 52],
                                      wqk[fb + cc * 128: fb + (cc + 1) * 128, :])
                    nc.sync.dma_start(wv_sb[:, cc * 64:(cc + 1) * 64],
                                      wv[fb + cc * 128: fb + (cc + 1) * 128, :])
                # host aux rows: q rows 16:20 = [0, 1, -s', 1]; k rows 16:20 = [1, s+b+c', 1, 0]
                nc.sync.dma_start(q_sb[16:20, :], aux[r * 8 + 0: r * 8 + 4, :])
                nc.sync.dma_start(k_sb[16:20, :], aux[r * 8 + 4: r * 8 + 8, :])
                v_view = v_sb.rearrange("p (c w) -> p c w", w=65)
                nc.vector.memset(v_view[:, :, 64], 1.0)

                for tt in range(NT):
                    t0 = tt * 512
                    # --- qk projection (o-major), psum rows: q 0:17, k 32:52
                    ps_qk = ps_qkp.tile([52, 512], mybir.dt.float32, tag="ps_qk")
                    for cc in range(3):
                        nc.tensor.matmul(
                            ps_qk[:, :],
                            _mm(wqk_sb[:, cc * 52:(cc + 1) * 52], "feat"),
                            _mm(ft[:, cc * N + t0: cc * N + t0 + 512], "feat"),
                            start=(cc == 0), stop=(cc == 2))
                    nc.vector.tensor_copy(q_sb[0:16, t0:t0 + 512], ps_qk[0:16, :])
                    nc.vector.tensor_copy(k_sb[0:16, t0:t0 + 512], ps_qk[32:48, :])
                    # --- v projection (t-major), 4 chunks of 128 tokens
                    ps_v = ps_vp.tile([128, 256], mybir.dt.float32, tag="ps_v")
                    for u in range(4):
                        tc0 = t0 + u * 128
                        for cc in range(3):
                            nc.tensor.matmul(
                                ps_v[:, u * 64:(u + 1) * 64],
                                _mm(ft[:, cc * N + tc0: cc * N + tc0 + 128], "feat"),
                                _mm(wv_sb[:, cc * 64:(cc + 1) * 64], "feat"),
                                start=(u == 0 and cc == 0), stop=(u == 3 and cc == 2))
                    ci0 = tt * 4
                    nc.vector.tensor_copy(
                        v_view[:, ci0:ci0 + 4, 0:64],
                        ps_v.rearrange("p (c w) -> p c w", w=64))

                # --- attention: cluster pairs
                for pp in range(K // 2):
                    kk0 = pp * 2
                    e_tiles = []
                    for jc in range(2):
                        ps_s = ps_sp.tile([128, 512], mybir.dt.float32, tag="ps_s")
                        for u in range(2):  # cluster kk0+u
                            col = (kk0 + u) * 256
                            nc.tensor.matmul(
                                ps_s[:, u * 256:(u + 1) * 256],
                                _mm(k_sb[0:20, col + jc * 128: col + (jc + 1) * 128], "qk"),
                                _mm(q_sb[0:20, col: col + 256], "qk"),
                                start=(u == 0), stop=(u == 1))
                        et = p_e.tile([128, 512], dt_e, tag="e")
                        nc.scalar.activation(et[:, :], ps_s[:, :],
                                             mybir.ActivationFunctionType.Exp)
                        e_tiles.append(et)
                    for u in range(2):
                        kk = kk0 + u
                        for ic in range(2):
                            ps_o = ps_op.tile([128, 65], mybir.dt.float32, tag="ps_o")
                            for jc in range(2):
                                nc.tensor.matmul(
                                    ps_o[:, :],
                                    _mm(e_tiles[jc][:, u * 256 + ic * 128:
                                                    u * 256 + (ic + 1) * 128], "e"),
                                    _mm(v_view[:, kk * 2 + jc, :], "e"),
                                    start=(jc == 0), stop=(jc == 1))
                            inv = p_out.tile([128, 1], mybir.dt.float32, tag="inv")
                            nc.vector.reciprocal(inv[:, :], ps_o[:, 64:65])
                            ot = p_out.tile([128, 64], mybir.dt.float32, tag="ot")
                            nc.vector.tensor_scalar_mul(ot[:, :], ps_o[:, 0:64], inv[:, :])
                            row0 = r * N + kk * 256 + ic * 128
                            nc.sync.dma_start(out_g[row0: row0 + 128, :], ot[:, :])
    nc.compile()
    return nc


def build_phase_b():
    dt_b = _DT[CFG["b"]]
    nc = bacc.Bacc(None, target_bir_lowering=False)
    f2T = nc.dram_tensor("f2T", [6 * 128, TPB], dt_b, kind="ExternalInput")
    wp2 = nc.dram_tensor("wp2", [6 * 128, 384], dt_b, kind="ExternalInput")
    bias = nc.dram_tensor("bias", [3 * 128, 1], mybir.dt.float32, kind="ExternalInput")
    outT = nc.dram_tensor("outT", [3 * 128, TPB], mybir.dt.float32, kind="ExternalOutput")

    with tile.TileContext(nc) as tc:
        with (
            tc.tile_pool(name="sb", bufs=1) as pool,
            tc.tile_pool(name="sb_o", bufs=4) as p_o,
            tc.tile_pool(name="ps", bufs=4, space="PSUM") as ps,
        ):
            fsb = pool.tile([128, 6 * TPB], dt_b, tag="fsb")
            wsb = pool.tile([128, 6 * 384], dt_b, tag="wsb")
            bsb = pool.tile([128, 3], mybir.dt.float32, tag="bsb")
            for cc in range(6):
                nc.sync.dma_start(fsb[:, cc * TPB:(cc + 1) * TPB],
                                  f2T[cc * 128:(cc + 1) * 128, :])
                nc.sync.dma_start(wsb[:, cc * 384:(cc + 1) * 384],
                                  wp2[cc * 128:(cc + 1) * 128, :])
            for oc in range(3):
                nc.sync.dma_start(bsb[:, oc:oc + 1], bias[oc * 128:(oc + 1) * 128, :])
            for tt in range(TPB // 512):
                t0 = tt * 512
                for oc in range(3):
                    p = ps.tile([128, 512], mybir.dt.float32, tag="p")
                    for cc in range(6):
                        nc.tensor.matmul(
                            p[:, :],
                            _mm(wsb[:, cc * 384 + oc * 128: cc * 384 + (oc + 1) * 128], "b"),
                            _mm(fsb[:, cc * TPB + t0: cc * TPB + t0 + 512], "b"),
                            start=(cc == 0), stop=(cc == 5))
                    osb = p_o.tile([128, 512], mybir.dt.float32, tag="osb")
                    nc.vector.tensor_scalar(osb[:, :], p[:, :], bsb[:, oc:oc + 1], None,
                                            mybir.AluOpType.add)
                    nc.sync.dma_start(outT[oc * 128:(oc + 1) * 128, t0:t0 + 512],
                                      osb[:, :])
    nc.compile()
    return nc


_CACHE = {}


def _get(name, builder):
    if name not in _CACHE:
        _CACHE[name] = builder()
    return _CACHE[name]


def kernel(pos, feat, member_idx, w_qkv, b_qkv, w_pos, b_pos, w_proj, b_proj):
    import time
    np_feat, np_qk, np_e, np_b = (_NP[CFG[k]] for k in ("feat", "qk", "e", "b"))
    pos = np.asarray(pos, np.float32)
    feat = np.asarray(feat, np.float32)
    mf = np.asarray(member_idx).astype(np.int64).reshape(BH, N)
    w_qkv = np.asarray(w_qkv, np.float32); b_qkv = np.asarray(b_qkv, np.float32)
    w_pos = np.asarray(w_pos, np.float32); b_pos = np.asarray(b_pos, np.float32)
    w_proj = np.asarray(w_proj, np.float32); b_proj = np.asarray(b_proj, np.float32)

    t_prep0 = time.time()
    # ---- host shard prep
    pos_n = pos / pos.reshape(-1, D).max(0)
    b_of = np.repeat(np.arange(B), H)
    pos_g = np.take_along_axis(pos_n[b_of], mf[:, :, None], axis=1)      # [48,N,2]
    s_all = np.einsum('rnd,rd->rn', pos_g, np.tile(w_pos, (B, 1)))       # [48,N]

    featbig = np.ascontiguousarray(feat.transpose(0, 2, 1))              # [B,C,N]
    featT_all = np.empty((BH, C, N), np.float32)
    for r in range(BH):
        np.take(featbig[b_of[r]], mf[r], axis=1, out=featT_all[r])

    wqk_all = np.zeros((BH, C, 52), np.float32)
    wv_all = np.empty((BH, C, 64), np.float32)
    aux_all = np.empty((BH, 8, N), np.float32)
    for h in range(H):
        Wq = w_qkv[h * 96: h * 96 + 16]
        Wk = w_qkv[h * 96 + 16: h * 96 + 32]
        Wv = w_qkv[h * 96 + 32: h * 96 + 96]
        bq = b_qkv[h * 96: h * 96 + 16]
        bk = b_qkv[h * 96 + 16: h * 96 + 32]
        has_bias = bool(np.any(bq) or np.any(bk))
        for b in range(B):
            r = b * H + h
            wqk_all[r, :, 0:16] = SCALE * Wq.T
            wqk_all[r, :, 32:48] = Wk.T
            wv_all[r] = Wv.T
            aux_all[r, 0] = 0.0
            aux_all[r, 1] = 1.0
            aux_all[r, 2] = -s_all[r]
            aux_all[r, 3] = 1.0
            aux_all[r, 4] = 1.0
            aux_all[r, 5] = s_all[r] + b_pos[h]
            aux_all[r, 6] = 1.0
            aux_all[r, 7] = 0.0
            if has_bias:
                # exact bias folding: logit = scale*(q+bq).(k+bk) + ...
                q_raw = Wq @ featT_all[r].reshape(C, N)
                k_raw = Wk @ featT_all[r].reshape(C, N)
                aux_all[r, 2] += SCALE * (bk @ q_raw)
                aux_all[r, 5] += SCALE * (bq @ k_raw) + SCALE * float(bq @ bk)

    in_maps_a = []
    for c in range(8):
        rs = slice(c * R, (c + 1) * R)
        in_maps_a.append({
            "featT": featT_all[rs].reshape(R * 3 * 128, N).astype(np_feat, copy=False),
            "wqk": wqk_all[rs].reshape(R * 3 * 128, 52).astype(np_feat, copy=False),
            "wv": wv_all[rs].reshape(R * 3 * 128, 64).astype(np_feat, copy=False),
            "aux": aux_all[rs].reshape(R * 8, N).astype(np_qk, copy=False),
        })
    t_prep1 = time.time()

    nc_a = _get("a", build_phase_a)
    t_run_a0 = time.time()
    res_a = run_bass_kernel_spmd(nc_a, in_maps_a, core_ids=list(range(8)))
    t_run_a1 = time.time()

    out_g_all = np.concatenate(
        [res_a.results[c]["out_g"].reshape(R, N, 64) for c in range(8)], axis=0)

    # ---- host scatter to token order, build feat2T
    out_gT = np.ascontiguousarray(out_g_all.transpose(0, 2, 1))          # [48,64,N]
    f2T = np.empty((B, 2 * C, N), np.float32)
    for r in range(BH):
        b, h = divmod(r, H)
        f2T[b, h * 64:(h + 1) * 64, mf[r]] = out_gT[r].T
    wp2 = np.ascontiguousarray(w_proj.T)                                  # [768,384]
    b_eff = b_proj + w_proj[:, :] @ np.concatenate(
        [b_qkv[h * 96 + 32: h * 96 + 96] for h in range(H)])
    in_maps_b = []
    for c in range(8):
        b, half = divmod(c, 2)
        tsl = slice(half * TPB, (half + 1) * TPB)
        in_maps_b.append({
            "f2T": f2T[b][:, tsl].astype(np_b, copy=False),
            "wp2": wp2.astype(np_b, copy=False),
            "bias": b_eff.reshape(384, 1).astype(np.float32, copy=False),
        })
    t_prep2 = time.time()

    nc_b = _get("b", build_phase_b)
    t_run_b0 = time.time()
    res_b = run_bass_kernel_spmd(nc_b, in_maps_b, core_ids=list(range(8)))
    t_run_b1 = time.time()

    out = np.empty((B, N, C), np.float32)
    for c in range(8):
        b, half = divmod(c, 2)
        out[b, half * TPB:(half + 1) * TPB, :] = res_b.results[c]["outT"].T
    if os.environ.get("KTIME"):
        print(f"[kernel] prep1={t_prep1-t_prep0:.2f}s runA={t_run_a1-t_run_a0:.2f}s "
              f"prep2={t_prep2-t_run_a1:.2f}s runB={t_run_b1-t_run_b0:.2f}s")
    return out


# revision 8
# speedup vs baseline: 25028.8953x; 25028.8953x over previous
"""ClusterAttention Trainium2 kernel.

Sharding: 48 (b*h) rows -> 6 rows per core (8 cores). Host gathers feat into
cluster order per row (transposed, c-major) as shard prep; device does all
matmul/softmax compute; host scatters head outputs back to token order between
the two device phases and sums nothing (phase B consumes all 12 heads per
token on one core).

Phase A (per core, per row r with head h):
  - o-major qk projection: psum[52,512] = wqk[cc].T @ featT tile, accumulated
    over 3 c-chunks. M-layout: rows 0:16 q*scale, 16 qA, 32:48 k, 51 kB.
  - q_sb[20,8192] rows: [q(16), qA, 1, -s, 1];  k_sb rows: [k(16), 1, s+b+c, 1, kB]
    so that sum_p q_aug[p]*k_aug[p] = scale*(q+bq).(k+bk) + s_j - s_i + b_pos.
  - t-major v projection: psum[128,256] = featT_chunk.T @ wv[cc], -> v_sb with a
    ones column per 64-col chunk (for the softmax denominator).
  - per cluster: S'[j,i] = k_aug.T@q_aug (K=20), E = exp(S'), AV: out[i,0:65] =
    sum_j E[j,i-chunk].T @ v_aug[j,0:65]; col 64 = denominator. Normalize by
    DVE reciprocal + per-partition scalar multiply. DMA out rows.

Phase B (per core): o-major projection outT[o,t] = w_proj chunks.T @ feat2T,
bias added via DVE tensor_scalar.
"""
import os
import numpy as np
import ml_dtypes

import concourse.bacc as bacc
import concourse.tile as tile
from concourse import mybir
from concourse.bass_utils import run_bass_kernel_spmd

B, N, C, H, D, K, M = 4, 8192, 384, 12, 2, 32, 256
CH = C // H // 2            # 16
BH = B * H                  # 48
R = BH // 8                 # 6 rows per core
SCALE = float((C // H) ** -0.5)
NT = N // 512               # 16 token tiles per row
TPB = N * B // 8            # 4096 tokens per core in phase B

_DT = {"f32": mybir.dt.float32, "bf16": mybir.dt.bfloat16, "f32r": mybir.dt.float32r}
_NP = {"f32": np.float32, "bf16": ml_dtypes.bfloat16, "f32r": np.float32}


def _parse_cfg():
    s = os.environ.get("KCFG", "feat=f32,qk=f32,e=f32,b=f32,r=0")
    cfg = {}
    for part in s.split(","):
        k, v = part.split("=")
        cfg[k] = v
    cfg.setdefault("feat", "f32"); cfg.setdefault("qk", "f32")
    cfg.setdefault("e", "f32"); cfg.setdefault("b", "f32")
    cfg.setdefault("r", "0")  # r=1: bitcast f32 matmul operands to float32r
    return cfg


CFG = _parse_cfg()


def _mm(ap, kind):
    # bitcast fp32 matmul inputs to float32r when enabled
    if CFG["r"] == "1" and CFG[kind] == "f32":
        return ap.bitcast(mybir.dt.float32r)
    return ap


def build_phase_a():
    dt_feat, dt_qk, dt_e = _DT[CFG["feat"]], _DT[CFG["qk"]], _DT[CFG["e"]]
    dt_wv = mybir.dt.float32 if CFG["feat"] == "f32r" else dt_feat
    nc = bacc.Bacc(None, target_bir_lowering=False)
    featT = nc.dram_tensor("featT", [R * 3 * 128, N], dt_feat, kind="ExternalInput")
    wqk = nc.dram_tensor("wqk", [R * 3 * 128, 52], dt_feat, kind="ExternalInput")
    wv = nc.dram_tensor("wv", [R * 3 * 128, 64], dt_wv, kind="ExternalInput")
    aux = nc.dram_tensor("aux", [R * 8, N], dt_qk, kind="ExternalInput")
    out_g = nc.dram_tensor("out_g", [R * N, 64], mybir.dt.float32, kind="ExternalOutput")

    with tile.TileContext(nc) as tc:
        with (
            tc.tile_pool(name="sb_feat", bufs=1) as p_feat,
            tc.tile_pool(name="sb_row", bufs=1) as p_row,
            tc.tile_pool(name="sb_w", bufs=2) as p_w,
            tc.tile_pool(name="sb_e", bufs=4) as p_e,
            tc.tile_pool(name="sb_out", bufs=4) as p_out,
            tc.tile_pool(name="ps_qkp", bufs=2, space="PSUM") as ps_qkp,
            tc.tile_pool(name="ps_sp", bufs=2, space="PSUM") as ps_sp,
            tc.tile_pool(name="ps_vp", bufs=2, space="PSUM") as ps_vp,
            tc.tile_pool(name="ps_op", bufs=2, space="PSUM") as ps_op,
        ):
            for r in range(R):
                fb = r * 3 * 128
                # per-row persistent tiles
                ft = p_feat.tile([128, 3 * N], dt_feat, tag="ft")
                q_sb = p_row.tile([20, N], dt_qk, tag="q_sb")
                k_sb = p_row.tile([20, N], dt_qk, tag="k_sb")
                v_sb = p_row.tile([128, 64 * 65], dt_e, tag="v_sb")
                wqk_sb = p_w.tile([128, 3 * 52], dt_feat, tag="wqk_sb")
                wv_sb = p_w.tile([128, 3 * 64], dt_wv, tag="wv_sb")
                for cc in range(3):
                    nc.sync.dma_start(ft[:, cc * N:(cc + 1) * N],
                                      featT[fb + cc * 128: fb + (cc + 1) * 128, :])
                    nc.sync.dma_start(wqk_sb[:, cc * 52:(cc + 1) * 52],
                                      wqk[fb + cc * 128: fb + (cc + 1) * 128, :])
                    nc.sync.dma_start(wv_sb[:, cc * 64:(cc + 1) * 64],
                                      wv[fb + cc * 128: fb + (cc + 1) * 128, :])
                # host aux rows: q rows 16:20 = [0, 1, -s', 1]; k rows 16:20 = [1, s+b+c', 1, 0]
                nc.sync.dma_start(q_sb[16:20, :], aux[r * 8 + 0: r * 8 + 4, :])
                nc.sync.dma_start(k_sb[16:20, :], aux[r * 8 + 4: r * 8 + 8, :])
                v_view = v_sb.rearrange("p (c w) -> p c w", w=65)
                nc.vector.memset(v_view[:, :, 64], 1.0)

                for tt in range(NT):
                    t0 = tt * 512
                    # --- qk projection (o-major), psum rows: q 0:17, k 32:52
                    ps_qk = ps_qkp.tile([52, 512], mybir.dt.float32, tag="ps_qk")
                    for cc in range(3):
                        nc.tensor.matmul(
                            ps_qk[:, :],
                            _mm(wqk_sb[:, cc * 52:(cc + 1) * 52], "feat"),
                            _mm(ft[:, cc * N + t0: cc * N + t0 + 512], "feat"),
                            start=(cc == 0), stop=(cc == 2))
                    nc.vector.tensor_copy(q_sb[0:16, t0:t0 + 512], ps_qk[0:16, :])
                    nc.vector.tensor_copy(k_sb[0:16, t0:t0 + 512], ps_qk[32:48, :])
                    # --- v projection (t-major), 4 chunks of 128 tokens
                    ps_v = ps_vp.tile([128, 256], mybir.dt.float32, tag="ps_v")
                    for u in range(4):
                        tc0 = t0 + u * 128
                        for cc in range(3):
                            ftv = ft[:, cc * N + tc0: cc * N + tc0 + 128]
                            if CFG["feat"] == "f32r":
                                ftv = ftv.bitcast(mybir.dt.float32)
                            nc.tensor.matmul(
                                ps_v[:, u * 64:(u + 1) * 64],
                                _mm(ftv, "feat"),
                                _mm(wv_sb[:, cc * 64:(cc + 1) * 64], "feat"),
                                start=(u == 0 and cc == 0), stop=(u == 3 and cc == 2))
                    ci0 = tt * 4
                    nc.vector.tensor_copy(
                        v_view[:, ci0:ci0 + 4, 0:64],
                        ps_v.rearrange("p (c w) -> p c w", w=64))

                # --- attention: cluster pairs
                for pp in range(K // 2):
                    kk0 = pp * 2
                    e_tiles = []
                    for jc in range(2):
                        ps_s = ps_sp.tile([128, 512], mybir.dt.float32, tag="ps_s")
                        for u in range(2):  # cluster kk0+u
                            col = (kk0 + u) * 256
                            nc.tensor.matmul(
                                ps_s[:, u * 256:(u + 1) * 256],
                                _mm(k_sb[0:20, col + jc * 128: col + (jc + 1) * 128], "qk"),
                                _mm(q_sb[0:20, col: col + 256], "qk"),
                                start=(u == 0), stop=(u == 1))
                        et = p_e.tile([128, 512], dt_e, tag="e")
                        nc.scalar.activation(et[:, :], ps_s[:, :],
                                             mybir.ActivationFunctionType.Exp)
                        e_tiles.append(et)
                    for u in range(2):
                        kk = kk0 + u
                        for ic in range(2):
                            ps_o = ps_op.tile([128, 65], mybir.dt.float32, tag="ps_o")
                            for jc in range(2):
                                nc.tensor.matmul(
                                    ps_o[:, :],
                                    _mm(e_tiles[jc][:, u * 256 + ic * 128:
                                                    u * 256 + (ic + 1) * 128], "e"),
                                    _mm(v_view[:, kk * 2 + jc, :], "e"),
                                    start=(jc == 0), stop=(jc == 1))
                            inv = p_out.tile([128, 1], mybir.dt.float32, tag="inv")
                            nc.vector.reciprocal(inv[:, :], ps_o[:, 64:65])
                            ot = p_out.tile([128, 64], mybir.dt.float32, tag="ot")
                            nc.vector.tensor_scalar_mul(ot[:, :], ps_o[:, 0:64], inv[:, :])
                            row0 = r * N + kk * 256 + ic * 128
                            nc.sync.dma_start(out_g[row0: row0 + 128, :], ot[:, :])
    nc.compile()
    return nc


def build_phase_b():
    dt_b = _DT[CFG["b"]]
    nc = bacc.Bacc(None, target_bir_lowering=False)
    f2T = nc.dram_tensor("f2T", [6 * 128, TPB], dt_b, kind="ExternalInput")
    wp2 = nc.dram_tensor("wp2", [6 * 128, 384], dt_b, kind="ExternalInput")
    bias = nc.dram_tensor("bias", [3 * 128, 1], mybir.dt.float32, kind="ExternalInput")
    outT = nc.dram_tensor("outT", [3 * 128, TPB], mybir.dt.float32, kind="ExternalOutput")

    with tile.TileContext(nc) as tc:
        with (
            tc.tile_pool(name="sb", bufs=1) as pool,
            tc.tile_pool(name="sb_o", bufs=4) as p_o,
            tc.tile_pool(name="ps", bufs=4, space="PSUM") as ps,
        ):
            fsb = pool.tile([128, 6 * TPB], dt_b, tag="fsb")
            wsb = pool.tile([128, 6 * 384], dt_b, tag="wsb")
            bsb = pool.tile([128, 3], mybir.dt.float32, tag="bsb")
            for cc in range(6):
                nc.sync.dma_start(fsb[:, cc * TPB:(cc + 1) * TPB],
                                  f2T[cc * 128:(cc + 1) * 128, :])
                nc.sync.dma_start(wsb[:, cc * 384:(cc + 1) * 384],
                                  wp2[cc * 128:(cc + 1) * 128, :])
            for oc in range(3):
                nc.sync.dma_start(bsb[:, oc:oc + 1], bias[oc * 128:(oc + 1) * 128, :])
            for tt in range(TPB // 512):
                t0 = tt * 512
                for oc in range(3):
                    p = ps.tile([128, 512], mybir.dt.float32, tag="p")
                    for cc in range(6):
                        nc.tensor.matmul(
                            p[:, :],
                            _mm(wsb[:, cc * 384 + oc * 128: cc * 384 + (oc + 1) * 128], "b"),
                            _mm(fsb[:, cc * TPB + t0: cc * TPB + t0 + 512], "b"),
                            start=(cc == 0), stop=(cc == 5))
                    osb = p_o.tile([128, 512], mybir.dt.float32, tag="osb")
                    nc.vector.tensor_scalar(osb[:, :], p[:, :], bsb[:, oc:oc + 1], None,
                                            mybir.AluOpType.add)
                    nc.sync.dma_start(outT[oc * 128:(oc + 1) * 128, t0:t0 + 512],
                                      osb[:, :])
    nc.compile()
    return nc


_CACHE = {}


def _get(name, builder):
    if name not in _CACHE:
        _CACHE[name] = builder()
    return _CACHE[name]


def kernel(pos, feat, member_idx, w_qkv, b_qkv, w_pos, b_pos, w_proj, b_proj):
    import time
    np_feat, np_qk, np_e, np_b = (_NP[CFG[k]] for k in ("feat", "qk", "e", "b"))
    pos = np.asarray(pos, np.float32)
    feat = np.asarray(feat, np.float32)
    mf = np.asarray(member_idx).astype(np.int64).reshape(BH, N)
    w_qkv = np.asarray(w_qkv, np.float32); b_qkv = np.asarray(b_qkv, np.float32)
    w_pos = np.asarray(w_pos, np.float32); b_pos = np.asarray(b_pos, np.float32)
    w_proj = np.asarray(w_proj, np.float32); b_proj = np.asarray(b_proj, np.float32)

    t_prep0 = time.time()
    # ---- host shard prep
    pos_n = pos / pos.reshape(-1, D).max(0)
    b_of = np.repeat(np.arange(B), H)
    pos_g = np.take_along_axis(pos_n[b_of], mf[:, :, None], axis=1)      # [48,N,2]
    s_all = np.einsum('rnd,rd->rn', pos_g, np.tile(w_pos, (B, 1)))       # [48,N]

    featbig = np.ascontiguousarray(feat.transpose(0, 2, 1))              # [B,C,N]
    featT_all = np.empty((BH, C, N), np.float32)
    for r in range(BH):
        np.take(featbig[b_of[r]], mf[r], axis=1, out=featT_all[r])

    wqk_all = np.zeros((BH, C, 52), np.float32)
    wv_all = np.empty((BH, C, 64), np.float32)
    aux_all = np.empty((BH, 8, N), np.float32)
    for h in range(H):
        Wq = w_qkv[h * 96: h * 96 + 16]
        Wk = w_qkv[h * 96 + 16: h * 96 + 32]
        Wv = w_qkv[h * 96 + 32: h * 96 + 96]
        bq = b_qkv[h * 96: h * 96 + 16]
        bk = b_qkv[h * 96 + 16: h * 96 + 32]
        has_bias = bool(np.any(bq) or np.any(bk))
        for b in range(B):
            r = b * H + h
            wqk_all[r, :, 0:16] = SCALE * Wq.T
            wqk_all[r, :, 32:48] = Wk.T
            wv_all[r] = Wv.T
            aux_all[r, 0] = 0.0
            aux_all[r, 1] = 1.0
            aux_all[r, 2] = -s_all[r]
            aux_all[r, 3] = 1.0
            aux_all[r, 4] = 1.0
            aux_all[r, 5] = s_all[r] + b_pos[h]
            aux_all[r, 6] = 1.0
            aux_all[r, 7] = 0.0
            if has_bias:
                # exact bias folding: logit = scale*(q+bq).(k+bk) + ...
                q_raw = Wq @ featT_all[r].reshape(C, N)
                k_raw = Wk @ featT_all[r].reshape(C, N)
                aux_all[r, 2] += SCALE * (bk @ q_raw)
                aux_all[r, 5] += SCALE * (bq @ k_raw) + SCALE * float(bq @ bk)

    in_maps_a = []
    for c in range(8):
        rs = slice(c * R, (c + 1) * R)
        in_maps_a.append({
            "featT": featT_all[rs].reshape(R * 3 * 128, N).astype(np_feat, copy=False),
            "wqk": wqk_all[rs].reshape(R * 3 * 128, 52).astype(np_feat, copy=False),
            "wv": wv_all[rs].reshape(R * 3 * 128, 64).astype(np_feat, copy=False),
            "aux": aux_all[rs].reshape(R * 8, N).astype(np_qk, copy=False),
        })
    t_prep1 = time.time()

    nc_a = _get("a", build_phase_a)
    t_run_a0 = time.time()
    res_a = run_bass_kernel_spmd(nc_a, in_maps_a, core_ids=list(range(8)))
    t_run_a1 = time.time()

    out_g_all = np.concatenate(
        [res_a.results[c]["out_g"].reshape(R, N, 64) for c in range(8)], axis=0)

    # ---- host scatter to token order, build feat2T
    out_gT = np.ascontiguousarray(out_g_all.transpose(0, 2, 1))          # [48,64,N]
    f2T = np.empty((B, 2 * C, N), np.float32)
    for r in range(BH):
        b, h = divmod(r, H)
        f2T[b, h * 64:(h + 1) * 64, mf[r]] = out_gT[r].T
    wp2 = np.ascontiguousarray(w_proj.T)                                  # [768,384]
    b_eff = b_proj + w_proj[:, :] @ np.concatenate(
        [b_qkv[h * 96 + 32: h * 96 + 96] for h in range(H)])
    in_maps_b = []
    for c in range(8):
        b, half = divmod(c, 2)
        tsl = slice(half * TPB, (half + 1) * TPB)
        in_maps_b.append({
            "f2T": f2T[b][:, tsl].astype(np_b, copy=False),
            "wp2": wp2.astype(np_b, copy=False),
            "bias": b_eff.reshape(384, 1).astype(np.float32, copy=False),
        })
    t_prep2 = time.time()

    nc_b = _get("b", build_phase_b)
    t_run_b0 = time.time()
    res_b = run_bass_kernel_spmd(nc_b, in_maps_b, core_ids=list(range(8)))
    t_run_b1 = time.time()

    out = np.empty((B, N, C), np.float32)
    for c in range(8):
        b, half = divmod(c, 2)
        out[b, half * TPB:(half + 1) * TPB, :] = res_b.results[c]["outT"].T
    if os.environ.get("KTIME"):
        print(f"[kernel] prep1={t_prep1-t_prep0:.2f}s runA={t_run_a1-t_run_a0:.2f}s "
              f"prep2={t_prep2-t_run_a1:.2f}s runB={t_run_b1-t_run_b0:.2f}s")
    return out
